# revision 18
# baseline (speedup 1.0000x reference)
"""Trainium2 Bass kernel for nn_FGNet (gnn_message_passing).

Strategy (v3, fp16)
-------------------
Per-edge weights are gathers from tiny tables (169 edge types), so edges are
sorted by type id and processed in uniform 256-edge blocks (one id per block,
padded; 2 segments x 128 edges).  All device data is fp16 (rel err ~3e-4,
gate is 2e-2), which halves HBM traffic vs f32 — the kernel is memory-bound.

Device math per block b (two blocks = one "pair" q share a 128-partition
feats tile: even block on partitions 0:64, odd on 64:128):

    ps1   = W_id.T @ feats          2 matmuls, K=64, N=384 each
                                    (odd block at PE tile_position (64,0) —
                                    row tiling, other half of the PE array)
    t     = relu(ps1 + b_id)        fp16, scalar engine (2/3) / DVE (1/3)
    p_i   = prod_{j != i} t_j       p0 -> DVE mul, p1/p2 -> gpsimd
                                    scalar_tensor_tensor ((x+0)*y)
    i01   = [ho_0|ho_1].T @ [p_0|p_1]   one matmul M=128 N=512: the
            off-diagonal quadrants (ho_1@p_0, ho_0@p_1) are garbage
    i2    = ho_2.T @ p_2            M=64 N=256, overwrites the upper
            garbage quadrant -> one dense PSUM bank per block:
            [0:64]=[msg0|msg2], [64:128]=[grbg|msg1]
    (second bias b2 is linear in the segment-sum -> folded to the host)

One copy per pair converts ps2 [128, 2, 512] f32 -> [128, 1024] fp16 and one
DMA stores it (the 25% garbage columns ride along; host discards them).
W / ho / bias tables for all the core's blocks are hoisted into SBUF once.

Host side (vectorized numpy): id computation, sort, feature gather, packing,
unpermute, b2 bias add and the final segment-sum into node_msg.
"""

import numpy as np

_BLK = 256          # edge slots per block (2 segments x 128)
_SEG = 128
_NCORES = 8

_prog_cache = {}


def _build_program(B):
    """Build the SPMD device program for B blocks per core (B even)."""
    import concourse.mybir as mybir
    import concourse.tile as tile
    from concourse import bacc

    F32 = mybir.dt.float32
    F16 = mybir.dt.float16
    Relu = mybir.ActivationFunctionType.Relu
    Copy = mybir.ActivationFunctionType.Copy
    Add = mybir.AluOpType.add
    Max = mybir.AluOpType.max
    Mult = mybir.AluOpType.mult

    assert B % 2 == 0
    PB = B // 2

    nc = bacc.Bacc()
    ft = nc.declare_dram_parameter("ft", [PB, 128, 768], F16, isOutput=False)
    wt = nc.declare_dram_parameter("wt", [128, PB * 128], F16, isOutput=False)
    ho = nc.declare_dram_parameter("ho", [128, B * 192], F16, isOutput=False)
    bia = nc.declare_dram_parameter("bia", [128, B], F32, isOutput=False)
    msgs = nc.declare_dram_parameter("msgs", [PB, 128, 1024], F16,
                                     isOutput=True)

    with tile.TileContext(nc) as tc:
        with (
            tc.tile_pool(name="const", bufs=1) as const,
            tc.tile_pool(name="work", bufs=2) as work,
            tc.tile_pool(name="psum", bufs=2, space="PSUM") as psum,
        ):
            wtt = const.tile([128, PB * 128], F16, name="wtt")
            hot = const.tile([128, B * 192], F16, name="hot")
            bt = const.tile([128, B], F32, name="bt")
            # startup order: transform needs wt+bias+ft0 first; ho is only
            # needed by the second stage and is loaded in block-aligned
            # chunks so early blocks don't wait on the whole table
            # wt in two chunks so pair 0's weights arrive first
            nc.sync.dma_start(out=wtt[:, 0:256], in_=wt[:, 0:256])
            nc.sync.dma_start(out=bt[:], in_=bia[:])

            PRE = 3            # ft prefetch depth (ft tag bufs)
            ftts = []
            for q in range(min(PRE, PB)):
                ftt = work.tile([128, 768], F16, name="ftt", tag="ft",
                                bufs=PRE)
                nc.sync.dma_start(out=ftt[:], in_=ft[q])
                ftts.append(ftt)
                if q == 0:
                    nc.sync.dma_start(out=wtt[:, 256:PB * 128],
                                      in_=wt[:, 256:PB * 128])
                    nc.sync.dma_start(out=hot[:, 0:4 * 192],
                                      in_=ho[:, 0:4 * 192])
            for c0 in range(4, B, 8):
                c1 = min(c0 + 8, B)
                nc.sync.dma_start(out=hot[:, c0 * 192:c1 * 192],
                                  in_=ho[:, c0 * 192:c1 * 192])



            # --- software-pipelined emission -------------------------------
            # The second stage of block b is emitted after the transforms of
            # block b+1, so by the time the PE reaches i01(b) the products
            # p[b] were computed while it streamed block b+1's transforms —
            # the PE never sits on the relu->mul chain of the current block.
            ps2s = {}
            ps = {}

            def pair_transforms(q):
                # interleave the two blocks' transform matmuls so adjacent
                # PE instructions target different array row-halves
                # (tile_position (0,0) vs (64,0)) and can overlap
                ftt = ftts[q]
                ps1s = [psum.tile([128, 2, 512], F32, name="ps1", tag="ps1")
                        for _ in range(2)]
                for s in range(2):
                    for par in range(2):
                        pl, ph = 64 * par, 64 * (par + 1)
                        nc.tensor.matmul(
                            out=ps1s[par][:, s, 0:384],
                            lhsT=wtt[pl:ph, q * 128:(q + 1) * 128],
                            rhs=ftt[pl:ph, 384 * s:384 * (s + 1)],
                            start=True, stop=True,
                        )
                return ps1s

            def elem(b, ps1):
                q, par = divmod(b, 2)
                # relu+bias split by segment across both engines
                t = work.tile([128, 2, 384], F16, name="t", tag="t", bufs=3)
                nc.scalar.activation(out=t[:, 0], in_=ps1[:, 0, 0:384],
                                     func=Relu, bias=bt[:, b:b + 1],
                                     scale=1.0)
                nc.vector.tensor_scalar(
                    out=t[:, 1], in0=ps1[:, 1, 0:384],
                    scalar1=bt[:, b:b + 1], scalar2=0.0,
                    op0=Add, op1=Max)

                p = work.tile([128, 3, 2, 128], F16, name="p", tag="p",
                              bufs=3)
                nc.vector.tensor_mul(out=p[:, 0], in0=t[:, :, 128:256],
                                     in1=t[:, :, 256:384])
                nc.gpsimd.tensor_mul(out=p[:, 1], in0=t[:, :, 0:128],
                                     in1=t[:, :, 256:384])
                nc.gpsimd.tensor_mul(out=p[:, 2], in0=t[:, :, 0:128],
                                     in1=t[:, :, 128:256])
                ps[b] = p

            def pair_second(qq):
                # interleave the two blocks' matmuls: consecutive PE
                # instructions hit different PSUM banks, so the fixed
                # SBUF-access latency of one overlaps the other's stream
                ps2s[qq] = psum.tile([128, 2, 512], F32, name="ps2",
                                     tag="ps2")
                ps2 = ps2s[qq]
                for par in range(2):
                    b = 2 * qq + par
                    base = b * 192
                    nc.tensor.matmul(
                        out=ps2[:, par, :],
                        lhsT=hot[:, base:base + 128],
                        rhs=ps[b][:, 0:2].rearrange("r i h e -> r (i h e)"),
                        start=True, stop=True)
                for par in range(2):
                    b = 2 * qq + par
                    base = b * 192
                    nc.tensor.matmul(
                        out=ps2[0:64, par, 256:512],
                        lhsT=hot[:, base + 128:base + 192],
                        rhs=ps[b][:, 2].rearrange("r h e -> r (h e)"),
                        start=True, stop=True)
                    del ps[b]

            def pair_out(q):
                ps2 = ps2s.pop(q)
                if q + PRE < PB:
                    nftt = work.tile([128, 768], F16, name="ftt", tag="ft",
                                     bufs=PRE)
                    nc.sync.dma_start(out=nftt[:], in_=ft[q + PRE])
                    ftts.append(nftt)
                mt = work.tile([128, 1024], F16, name="mt", tag="m")
                if q == PB - 1:
                    # tail: per-bank copy+store so the even block's results
                    # stream out while the odd block is still computing
                    nc.scalar.activation(out=mt[:, 0:512], in_=ps2[:, 0],
                                         func=Copy, bias=0.0, scale=1.0)
                    nc.sync.dma_start(out=msgs[q, :, 0:512], in_=mt[:, 0:512])
                    nc.vector.tensor_copy(out=mt[:, 512:1024], in_=ps2[:, 1])
                    nc.sync.dma_start(out=msgs[q, :, 512:1024],
                                      in_=mt[:, 512:1024])
                else:
                    nc.scalar.activation(
                        out=mt[:], in_=ps2[:].rearrange("p a b -> p (a b)"),
                        func=Copy, bias=0.0, scale=1.0)
                    nc.sync.dma_start(out=msgs[q], in_=mt[:])

            for q in range(PB):
                ps1s = pair_transforms(q)
                elem(2 * q, ps1s[0])
                elem(2 * q + 1, ps1s[1])
                if q >= 1:
                    pair_second(q - 1)
                    pair_out(q - 1)
            pair_second(PB - 1)
            pair_out(PB - 1)
    nc.finalize()
    return nc


def _get_program(B):
    if B not in _prog_cache:
        _prog_cache[B] = _build_program(B)
    return _prog_cache[B]


def _prepare(x, nodes, fact, params, bias_p, ho_params, ho_bias):
    """Host-side: sort by id, build per-block packed arrays (fp16)."""
    N, L = nodes.shape
    E = fact.shape[0]
    R = params.shape[2]
    NP = params.shape[0]           # 169
    MA = int(round(NP ** 0.5))     # 13

    ids = (x[fact[:, 0], 1] * MA + x[fact[:, 0], 2]).astype(np.int64)   # [E]
    perm = np.argsort(ids, kind="stable")
    ids_s = ids[perm]
    fact_s = fact[perm].astype(np.int64)                                 # [E,3]

    counts = np.bincount(ids_s, minlength=NP)                            # [NP]
    nblk = (counts + _BLK - 1) // _BLK                                   # [NP]
    blk_ids = np.repeat(np.arange(NP), nblk)                             # [NB]
    NB = int(blk_ids.shape[0])
    B = (NB + _NCORES - 1) // _NCORES
    if B % 2:
        B += 1
    NB8 = B * _NCORES
    PB = B // 2
    Q = NB8 // 2
    blk_ids = np.concatenate([blk_ids, np.zeros(NB8 - NB, np.int64)])

    # slot -> sorted-edge-position map (-1 = padding)
    padded = nblk * _BLK
    pad_off = np.concatenate([[0], np.cumsum(padded)])
    off = np.concatenate([[0], np.cumsum(counts)])
    total = int(pad_off[-1])
    t_of = np.repeat(np.arange(NP), padded)
    jloc = np.arange(total) - pad_off[t_of]
    src = np.where(jloc < counts[t_of], off[t_of] + jloc, -1)
    src = np.concatenate([src, np.full(NB8 * _BLK - total, -1, np.int64)])
    valid = src >= 0

    # gather features per slot (fp16)
    nf = nodes[fact_s].astype(np.float16)                                # [E,3,L]
    featp = np.zeros((NB8 * _BLK, 3, L), np.float16)
    featp[valid] = nf[src[valid]]

    # ft: [Q, 128, 768], partition = 64*par + l, col = s*384 + i*128 + e
    ftp = (
        featp.reshape(NB8, 2, _SEG, 3, L)        # b, s, e, i, l
        .transpose(0, 4, 1, 3, 2)                # b, l, s, i, e
        .reshape(Q, 2, L, 768)                   # q, par, l, col
        .reshape(Q, 128, 768)
    )

    # wt: [8, 128, PB*128], partition = 64*par + l, col = q*128 + r
    W = params[blk_ids].astype(np.float16)                               # [NB8,L,R]
    wtp = (
        W.reshape(_NCORES, PB, 2, L, R)          # c, q, par, l, r
        .transpose(0, 2, 3, 1, 4)                # c, par, l, q, r
        .reshape(_NCORES, 128, PB * R)
    )

    # ho: [8, 128, B*192], row r, col = b*192 + i*64 + l
    hop = (
        ho_params[:, blk_ids].astype(np.float16)  # i, b, r, l
        .transpose(1, 2, 0, 3)                    # b, r, i, l
        .reshape(_NCORES, B, R, 192)
        .transpose(0, 2, 1, 3)                    # c, r, b, (i l)
        .reshape(_NCORES, R, B * 192)
    )

    biasT = bias_p[blk_ids, 0].astype(np.float32)                        # [NB8,R]
    biasT = biasT.reshape(_NCORES, B, R).transpose(0, 2, 1)              # [8,R,B]

    return dict(ftp=np.ascontiguousarray(ftp),
                wtp=np.ascontiguousarray(wtp),
                hop=np.ascontiguousarray(hop),
                biasT=np.ascontiguousarray(biasT),
                B=B, NB8=NB8, Q=Q,
                src=src, valid=valid, fact_s=fact_s, ids_s=ids_s,
                N=N, E=E, L=L)


def _postprocess(msgs_all, prep, ho_bias):
    """Decode per-slot messages, add host-side b2, segment-sum into node_msg."""
    NB8, N, E, L = prep["NB8"], prep["N"], prep["E"], prep["L"]
    Q = prep["Q"]
    src, valid, fact_s, ids_s = prep["src"], prep["valid"], prep["fact_s"], prep["ids_s"]

    # msgs_all [Q, 128, 1024]: row = 64*rh + l, col = par*512 + c
    # block b=2q+par: msg0 = [rh=0, c 0:256], msg2 = [rh=0, c 256:512],
    #                 msg1 = [rh=1, c 256:512]; [rh=1, c 0:256] is garbage.
    arr = msgs_all.astype(np.float32).reshape(Q, 2, 64, 2, 512)  # q,rh,l,par,c
    m0 = arr[:, 0, :, :, 0:256]          # q,l,par,(s e)
    m1 = arr[:, 1, :, :, 256:512]
    m2 = arr[:, 0, :, :, 256:512]
    msgs_i = np.stack([m0, m1, m2], axis=2)       # q,l,i,par,(s e)
    slots = (
        msgs_i.reshape(Q, L, 3, 2, 2, _SEG)       # q,l,i,par,s,e
        .transpose(0, 3, 4, 5, 2, 1)              # q,par,s,e,i,l
        .reshape(NB8 * _BLK, 3, L)
    )

    msg_e = np.empty((E, 3, L), np.float32)
    msg_e[src[valid]] = slots[valid]

    # fold in the second bias (linear in the segment-sum)
    msg_e += ho_bias[:, ids_s, 0].astype(np.float32).transpose(1, 0, 2)  # [E,3,L]

    idx_all = fact_s.T.reshape(-1)                                       # [3E]
    val_all = msg_e.transpose(1, 0, 2).reshape(-1, L)                    # [3E,L]
    order = np.argsort(idx_all, kind="stable")
    idx_sorted = idx_all[order]
    val_sorted = val_all[order]
    uniq, starts = np.unique(idx_sorted, return_index=True)
    sums = np.add.reduceat(val_sorted, starts, axis=0)
    out = np.zeros((N, L), np.float32)
    out[uniq] = sums
    return out


def _run_device(prep, trace=False, trace_kwargs=None):
    from concourse.bass_utils import run_bass_kernel_spmd

    B = prep["B"]
    PB = B // 2
    nc = _get_program(B)
    in_maps = []
    for c in range(_NCORES):
        in_maps.append({
            "ft": prep["ftp"][c * PB:(c + 1) * PB],
            "wt": prep["wtp"][c],
            "ho": prep["hop"][c],
            "bia": prep["biasT"][c],
        })
    kwargs = {}
    if trace:
        kwargs["trace"] = True
        if trace_kwargs:
            kwargs.update(trace_kwargs)
    res = run_bass_kernel_spmd(nc, in_maps, list(range(_NCORES)), **kwargs)
    msgs_all = np.concatenate([res.results[c]["msgs"] for c in range(_NCORES)],
                              axis=0)
    return msgs_all, res


def kernel(x, nodes, fact, fact_dim, params, bias_p, ho_params, ho_bias,
           _trace=False, _trace_kwargs=None):
    x = np.asarray(x)
    nodes = np.asarray(nodes, dtype=np.float32)
    fact = np.asarray(fact)
    params = np.asarray(params)
    bias_p = np.asarray(bias_p)
    ho_params = np.asarray(ho_params)
    ho_bias = np.asarray(ho_bias)

    prep = _prepare(x, nodes, fact, params, bias_p, ho_params, ho_bias)
    msgs_all, res = _run_device(prep, trace=_trace, trace_kwargs=_trace_kwargs)
    out = _postprocess(msgs_all, prep, ho_bias)
    kernel.last_results = res
    return out


# revision 19
# speedup vs baseline: 1.0185x; 1.0185x over previous
"""Trainium2 Bass kernel for nn_FGNet (gnn_message_passing).

Strategy (v3, fp16)
-------------------
Per-edge weights are gathers from tiny tables (169 edge types), so edges are
sorted by type id and processed in uniform 256-edge blocks (one id per block,
padded; 2 segments x 128 edges).  All device data is fp16 (rel err ~3e-4,
gate is 2e-2), which halves HBM traffic vs f32 — the kernel is memory-bound.

Device math per block b (two blocks = one "pair" q share a 128-partition
feats tile: even block on partitions 0:64, odd on 64:128):

    ps1   = W_id.T @ feats          2 matmuls, K=64, N=384 each
                                    (odd block at PE tile_position (64,0) —
                                    row tiling, other half of the PE array)
    t     = relu(ps1 + b_id)        fp16, scalar engine (2/3) / DVE (1/3)
    p_i   = prod_{j != i} t_j       p0 -> DVE mul, p1/p2 -> gpsimd
                                    scalar_tensor_tensor ((x+0)*y)
    i01   = [ho_0|ho_1].T @ [p_0|p_1]   one matmul M=128 N=512: the
            off-diagonal quadrants (ho_1@p_0, ho_0@p_1) are garbage
    i2    = ho_2.T @ p_2            M=64 N=256, overwrites the upper
            garbage quadrant -> one dense PSUM bank per block:
            [0:64]=[msg0|msg2], [64:128]=[grbg|msg1]
    (second bias b2 is linear in the segment-sum -> folded to the host)

One copy per pair converts ps2 [128, 2, 512] f32 -> [128, 1024] fp16 and one
DMA stores it (the 25% garbage columns ride along; host discards them).
W / ho / bias tables for all the core's blocks are hoisted into SBUF once.

Host side (vectorized numpy): id computation, sort, feature gather, packing,
unpermute, b2 bias add and the final segment-sum into node_msg.
"""

import numpy as np

_BLK = 256          # edge slots per block (2 segments x 128)
_SEG = 128
_NCORES = 8

_prog_cache = {}


def _build_program(B):
    """Build the SPMD device program for B blocks per core (B even)."""
    import concourse.mybir as mybir
    import concourse.tile as tile
    from concourse import bacc

    F32 = mybir.dt.float32
    F16 = mybir.dt.float16
    Relu = mybir.ActivationFunctionType.Relu
    Copy = mybir.ActivationFunctionType.Copy
    Add = mybir.AluOpType.add
    Max = mybir.AluOpType.max
    Mult = mybir.AluOpType.mult

    assert B % 2 == 0
    PB = B // 2

    nc = bacc.Bacc()
    ft = nc.declare_dram_parameter("ft", [PB, 128, 768], F16, isOutput=False)
    wt = nc.declare_dram_parameter("wt", [128, PB * 128], F16, isOutput=False)
    ho = nc.declare_dram_parameter("ho", [128, B * 192], F16, isOutput=False)
    bia = nc.declare_dram_parameter("bia", [128, B], F32, isOutput=False)
    msgs = nc.declare_dram_parameter("msgs", [PB, 128, 1024], F16,
                                     isOutput=True)

    with tile.TileContext(nc) as tc:
        with (
            tc.tile_pool(name="const", bufs=1) as const,
            tc.tile_pool(name="work", bufs=2) as work,
            tc.tile_pool(name="psum", bufs=2, space="PSUM") as psum,
        ):
            wtt = const.tile([128, PB * 128], F16, name="wtt")
            hot = const.tile([128, B * 192], F16, name="hot")
            bt = const.tile([128, B], F32, name="bt")
            # startup order: transform needs wt+bias+ft0 first; ho is only
            # needed by the second stage and is loaded in block-aligned
            # chunks so early blocks don't wait on the whole table
            # wt in two chunks so pair 0's weights arrive first
            nc.sync.dma_start(out=wtt[:, 0:256], in_=wt[:, 0:256])
            nc.sync.dma_start(out=bt[:], in_=bia[:])

            PRE = 3            # ft prefetch depth (ft tag bufs)
            ftts = []
            for q in range(min(PRE, PB)):
                ftt = work.tile([128, 768], F16, name="ftt", tag="ft",
                                bufs=PRE)
                nc.sync.dma_start(out=ftt[:], in_=ft[q])
                ftts.append(ftt)
                if q == 0:
                    nc.sync.dma_start(out=wtt[:, 256:PB * 128],
                                      in_=wt[:, 256:PB * 128])
                    nc.sync.dma_start(out=hot[:, 0:4 * 192],
                                      in_=ho[:, 0:4 * 192])
            for c0 in range(4, B, 8):
                c1 = min(c0 + 8, B)
                nc.sync.dma_start(out=hot[:, c0 * 192:c1 * 192],
                                  in_=ho[:, c0 * 192:c1 * 192])



            # --- software-pipelined emission -------------------------------
            # The second stage of block b is emitted after the transforms of
            # block b+1, so by the time the PE reaches i01(b) the products
            # p[b] were computed while it streamed block b+1's transforms —
            # the PE never sits on the relu->mul chain of the current block.
            ps2s = {}
            ps = {}

            def pair_transforms(q):
                # interleave the two blocks' transform matmuls so adjacent
                # PE instructions target different array row-halves
                # (tile_position (0,0) vs (64,0)) and can overlap
                ftt = ftts[q]
                ps1s = [psum.tile([128, 2, 512], F32, name="ps1", tag="ps1")
                        for _ in range(2)]
                for s in range(2):
                    for par in range(2):
                        pl, ph = 64 * par, 64 * (par + 1)
                        nc.tensor.matmul(
                            out=ps1s[par][:, s, 0:384],
                            lhsT=wtt[pl:ph, q * 128:(q + 1) * 128],
                            rhs=ftt[pl:ph, 384 * s:384 * (s + 1)],
                            start=True, stop=True,
                        )
                return ps1s

            def elem(b, ps1):
                q, par = divmod(b, 2)
                # relu+bias split by segment across both engines
                t = work.tile([128, 2, 384], F16, name="t", tag="t", bufs=3)
                nc.scalar.activation(out=t[:, 0], in_=ps1[:, 0, 0:384],
                                     func=Relu, bias=bt[:, b:b + 1],
                                     scale=1.0)
                nc.vector.tensor_scalar(
                    out=t[:, 1], in0=ps1[:, 1, 0:384],
                    scalar1=bt[:, b:b + 1], scalar2=0.0,
                    op0=Add, op1=Max)

                p = work.tile([128, 3, 2, 128], F16, name="p", tag="p",
                              bufs=3)
                nc.vector.tensor_mul(out=p[:, 0], in0=t[:, :, 128:256],
                                     in1=t[:, :, 256:384])
                nc.gpsimd.tensor_mul(out=p[:, 1], in0=t[:, :, 0:128],
                                     in1=t[:, :, 256:384])
                nc.gpsimd.tensor_mul(out=p[:, 2], in0=t[:, :, 0:128],
                                     in1=t[:, :, 128:256])
                ps[b] = p

            def pair_second(qq):
                ps2s[qq] = psum.tile([128, 2, 512], F32, name="ps2",
                                     tag="ps2")
                ps2 = ps2s[qq]
                for par in range(2):
                    b = 2 * qq + par
                    base = b * 192
                    p = ps.pop(b)
                    nc.tensor.matmul(
                        out=ps2[:, par, :],
                        lhsT=hot[:, base:base + 128],
                        rhs=p[:, 0:2].rearrange("r i h e -> r (i h e)"),
                        start=True, stop=True)
                    nc.tensor.matmul(
                        out=ps2[0:64, par, 256:512],
                        lhsT=hot[:, base + 128:base + 192],
                        rhs=p[:, 2].rearrange("r h e -> r (h e)"),
                        start=True, stop=True)

            def pair_out(q):
                ps2 = ps2s.pop(q)
                if q + PRE < PB:
                    nftt = work.tile([128, 768], F16, name="ftt", tag="ft",
                                     bufs=PRE)
                    nc.sync.dma_start(out=nftt[:], in_=ft[q + PRE])
                    ftts.append(nftt)
                mt = work.tile([128, 1024], F16, name="mt", tag="m")
                if q == PB - 1:
                    # tail: per-bank copy+store so the even block's results
                    # stream out while the odd block is still computing
                    nc.scalar.activation(out=mt[:, 0:512], in_=ps2[:, 0],
                                         func=Copy, bias=0.0, scale=1.0)
                    nc.sync.dma_start(out=msgs[q, :, 0:512], in_=mt[:, 0:512])
                    nc.vector.tensor_copy(out=mt[:, 512:1024], in_=ps2[:, 1])
                    nc.sync.dma_start(out=msgs[q, :, 512:1024],
                                      in_=mt[:, 512:1024])
                else:
                    nc.scalar.activation(
                        out=mt[:], in_=ps2[:].rearrange("p a b -> p (a b)"),
                        func=Copy, bias=0.0, scale=1.0)
                    nc.sync.dma_start(out=msgs[q], in_=mt[:])

            for q in range(PB):
                ps1s = pair_transforms(q)
                elem(2 * q, ps1s[0])
                elem(2 * q + 1, ps1s[1])
                if q >= 1:
                    pair_second(q - 1)
                    pair_out(q - 1)
            pair_second(PB - 1)
            pair_out(PB - 1)
    nc.finalize()
    return nc


def _get_program(B):
    if B not in _prog_cache:
        _prog_cache[B] = _build_program(B)
    return _prog_cache[B]


def _prepare(x, nodes, fact, params, bias_p, ho_params, ho_bias):
    """Host-side: sort by id, build per-block packed arrays (fp16)."""
    N, L = nodes.shape
    E = fact.shape[0]
    R = params.shape[2]
    NP = params.shape[0]           # 169
    MA = int(round(NP ** 0.5))     # 13

    ids = (x[fact[:, 0], 1] * MA + x[fact[:, 0], 2]).astype(np.int64)   # [E]
    perm = np.argsort(ids, kind="stable")
    ids_s = ids[perm]
    fact_s = fact[perm].astype(np.int64)                                 # [E,3]

    counts = np.bincount(ids_s, minlength=NP)                            # [NP]
    nblk = (counts + _BLK - 1) // _BLK                                   # [NP]
    blk_ids = np.repeat(np.arange(NP), nblk)                             # [NB]
    NB = int(blk_ids.shape[0])
    B = (NB + _NCORES - 1) // _NCORES
    if B % 2:
        B += 1
    NB8 = B * _NCORES
    PB = B // 2
    Q = NB8 // 2
    blk_ids = np.concatenate([blk_ids, np.zeros(NB8 - NB, np.int64)])

    # slot -> sorted-edge-position map (-1 = padding)
    padded = nblk * _BLK
    pad_off = np.concatenate([[0], np.cumsum(padded)])
    off = np.concatenate([[0], np.cumsum(counts)])
    total = int(pad_off[-1])
    t_of = np.repeat(np.arange(NP), padded)
    jloc = np.arange(total) - pad_off[t_of]
    src = np.where(jloc < counts[t_of], off[t_of] + jloc, -1)
    src = np.concatenate([src, np.full(NB8 * _BLK - total, -1, np.int64)])
    valid = src >= 0

    # gather features per slot (fp16)
    nf = nodes[fact_s].astype(np.float16)                                # [E,3,L]
    featp = np.zeros((NB8 * _BLK, 3, L), np.float16)
    featp[valid] = nf[src[valid]]

    # ft: [Q, 128, 768], partition = 64*par + l, col = s*384 + i*128 + e
    ftp = (
        featp.reshape(NB8, 2, _SEG, 3, L)        # b, s, e, i, l
        .transpose(0, 4, 1, 3, 2)                # b, l, s, i, e
        .reshape(Q, 2, L, 768)                   # q, par, l, col
        .reshape(Q, 128, 768)
    )

    # wt: [8, 128, PB*128], partition = 64*par + l, col = q*128 + r
    W = params[blk_ids].astype(np.float16)                               # [NB8,L,R]
    wtp = (
        W.reshape(_NCORES, PB, 2, L, R)          # c, q, par, l, r
        .transpose(0, 2, 3, 1, 4)                # c, par, l, q, r
        .reshape(_NCORES, 128, PB * R)
    )

    # ho: [8, 128, B*192], row r, col = b*192 + i*64 + l
    hop = (
        ho_params[:, blk_ids].astype(np.float16)  # i, b, r, l
        .transpose(1, 2, 0, 3)                    # b, r, i, l
        .reshape(_NCORES, B, R, 192)
        .transpose(0, 2, 1, 3)                    # c, r, b, (i l)
        .reshape(_NCORES, R, B * 192)
    )

    biasT = bias_p[blk_ids, 0].astype(np.float32)                        # [NB8,R]
    biasT = biasT.reshape(_NCORES, B, R).transpose(0, 2, 1)              # [8,R,B]

    return dict(ftp=np.ascontiguousarray(ftp),
                wtp=np.ascontiguousarray(wtp),
                hop=np.ascontiguousarray(hop),
                biasT=np.ascontiguousarray(biasT),
                B=B, NB8=NB8, Q=Q,
                src=src, valid=valid, fact_s=fact_s, ids_s=ids_s,
                N=N, E=E, L=L)


def _postprocess(msgs_all, prep, ho_bias):
    """Decode per-slot messages, add host-side b2, segment-sum into node_msg."""
    NB8, N, E, L = prep["NB8"], prep["N"], prep["E"], prep["L"]
    Q = prep["Q"]
    src, valid, fact_s, ids_s = prep["src"], prep["valid"], prep["fact_s"], prep["ids_s"]

    # msgs_all [Q, 128, 1024]: row = 64*rh + l, col = par*512 + c
    # block b=2q+par: msg0 = [rh=0, c 0:256], msg2 = [rh=0, c 256:512],
    #                 msg1 = [rh=1, c 256:512]; [rh=1, c 0:256] is garbage.
    arr = msgs_all.astype(np.float32).reshape(Q, 2, 64, 2, 512)  # q,rh,l,par,c
    m0 = arr[:, 0, :, :, 0:256]          # q,l,par,(s e)
    m1 = arr[:, 1, :, :, 256:512]
    m2 = arr[:, 0, :, :, 256:512]
    msgs_i = np.stack([m0, m1, m2], axis=2)       # q,l,i,par,(s e)
    slots = (
        msgs_i.reshape(Q, L, 3, 2, 2, _SEG)       # q,l,i,par,s,e
        .transpose(0, 3, 4, 5, 2, 1)              # q,par,s,e,i,l
        .reshape(NB8 * _BLK, 3, L)
    )

    msg_e = np.empty((E, 3, L), np.float32)
    msg_e[src[valid]] = slots[valid]

    # fold in the second bias (linear in the segment-sum)
    msg_e += ho_bias[:, ids_s, 0].astype(np.float32).transpose(1, 0, 2)  # [E,3,L]

    idx_all = fact_s.T.reshape(-1)                                       # [3E]
    val_all = msg_e.transpose(1, 0, 2).reshape(-1, L)                    # [3E,L]
    order = np.argsort(idx_all, kind="stable")
    idx_sorted = idx_all[order]
    val_sorted = val_all[order]
    uniq, starts = np.unique(idx_sorted, return_index=True)
    sums = np.add.reduceat(val_sorted, starts, axis=0)
    out = np.zeros((N, L), np.float32)
    out[uniq] = sums
    return out


def _run_device(prep, trace=False, trace_kwargs=None):
    from concourse.bass_utils import run_bass_kernel_spmd

    B = prep["B"]
    PB = B // 2
    nc = _get_program(B)
    in_maps = []
    for c in range(_NCORES):
        in_maps.append({
            "ft": prep["ftp"][c * PB:(c + 1) * PB],
            "wt": prep["wtp"][c],
            "ho": prep["hop"][c],
            "bia": prep["biasT"][c],
        })
    kwargs = {}
    if trace:
        kwargs["trace"] = True
        if trace_kwargs:
            kwargs.update(trace_kwargs)
    res = run_bass_kernel_spmd(nc, in_maps, list(range(_NCORES)), **kwargs)
    msgs_all = np.concatenate([res.results[c]["msgs"] for c in range(_NCORES)],
                              axis=0)
    return msgs_all, res


def kernel(x, nodes, fact, fact_dim, params, bias_p, ho_params, ho_bias,
           _trace=False, _trace_kwargs=None):
    x = np.asarray(x)
    nodes = np.asarray(nodes, dtype=np.float32)
    fact = np.asarray(fact)
    params = np.asarray(params)
    bias_p = np.asarray(bias_p)
    ho_params = np.asarray(ho_params)
    ho_bias = np.asarray(ho_bias)

    prep = _prepare(x, nodes, fact, params, bias_p, ho_params, ho_bias)
    msgs_all, res = _run_device(prep, trace=_trace, trace_kwargs=_trace_kwargs)
    out = _postprocess(msgs_all, prep, ho_bias)
    kernel.last_results = res
    return out


# revision 21
# speedup vs baseline: 1.0215x; 1.0029x over previous
"""Trainium2 Bass kernel for nn_FGNet (gnn_message_passing).

Strategy (fp16, ~57us vs 94us f32r baseline)
--------------------------------------------
Per-edge weights are gathers from tiny tables (169 edge types), so edges are
sorted by type id and processed in uniform 256-edge blocks (one id per block,
padded; 2 segments x 128 edges).  All device data is fp16 (rel err ~3e-4,
gate is 2e-2), which halves HBM traffic vs f32 — the kernel was memory-bound
at f32 and is now jointly PE / DVE / DMA-bound.

Device math per block b (two blocks = one "pair" q share a 128-partition
feats tile: even block on partitions 0:64, odd on 64:128):

    ps1   = W_id.T @ feats          2 matmuls, K=64, N=384 each
                                    (odd block at PE tile_position (64,0) —
                                    row tiling, other half of the PE array)
    t     = relu(ps1 + b_id)        fp16; split by segment: scalar engine
                                    does s0, DVE does s1, concurrently
    p_i   = prod_{j != i} t_j       p0 -> DVE mul, p1/p2 -> gpsimd mul
    i01   = [ho_0|ho_1].T @ [p_0|p_1]   one matmul M=128 N=512: the
            off-diagonal quadrants (ho_1@p_0, ho_0@p_1) are garbage
    i2    = ho_2.T @ p_2            M=64 N=256, overwrites the upper
            garbage quadrant -> one dense PSUM bank per block:
            [0:64]=[msg0|msg2], [64:128]=[grbg|msg1]
    (second bias b2 is linear in the segment-sum -> folded to the host)

The emission is software-pipelined: the second stage of pair q is emitted
after the transforms of pair q+1 so the PE doesn't wait on the relu->mul
chain of the current pair.  One copy per pair converts ps2 [128, 2, 512]
f32 -> [128, 1024] fp16 and one DMA stores it (the 25% garbage columns ride
along; host discards them).  W / ho / bias tables for all the core's blocks
are hoisted into SBUF once, ordered so pair 0's weights land first.

Empirical notes (HW-measured on this stack):
  - f32r matmuls at N>=256 stream at the same 1 cyc/col as fp16 — fp16's
    win is DMA bytes, not PE speed; per-MM cost is ~350ns fixed
    (LDW + SBUF access latency + dispatch) + 0.417ns/col (PE is warm;
    HAM warmup probes showed no cold->warm transition).
  - gpsimd tensor ops run at 0.42x roofline (software DSP) and cannot
    touch PSUM; scalar_tensor_tensor doesn't exist on gpsimd.
  - matmul output cannot span a PSUM bank (N<=512 f32).
  - DVE fp16 tensor_tensor hits the 2x mode (~291ns/256 cols idle) but
    degrades to ~700ns under full-kernel SBUF contention.

Host side (vectorized numpy): id computation, sort, feature gather, packing,
unpermute, b2 bias add and the final segment-sum into node_msg.
"""

import numpy as np

_BLK = 256          # edge slots per block (2 segments x 128)
_SEG = 128
_NCORES = 8

_prog_cache = {}


def _build_program(B):
    """Build the SPMD device program for B blocks per core (B even)."""
    import concourse.mybir as mybir
    import concourse.tile as tile
    from concourse import bacc

    F32 = mybir.dt.float32
    F16 = mybir.dt.float16
    Relu = mybir.ActivationFunctionType.Relu
    Copy = mybir.ActivationFunctionType.Copy
    Add = mybir.AluOpType.add
    Max = mybir.AluOpType.max

    assert B % 2 == 0
    PB = B // 2

    nc = bacc.Bacc()
    ft = nc.declare_dram_parameter("ft", [PB, 128, 768], F16, isOutput=False)
    wt = nc.declare_dram_parameter("wt", [128, PB * 128], F16, isOutput=False)
    ho = nc.declare_dram_parameter("ho", [128, B * 192], F16, isOutput=False)
    bia = nc.declare_dram_parameter("bia", [128, B], F32, isOutput=False)
    msgs = nc.declare_dram_parameter("msgs", [PB, 128, 1024], F16,
                                     isOutput=True)

    with tile.TileContext(nc) as tc:
        with (
            tc.tile_pool(name="const", bufs=1) as const,
            tc.tile_pool(name="work", bufs=2) as work,
            tc.tile_pool(name="psum", bufs=2, space="PSUM") as psum,
        ):
            wtt = const.tile([128, PB * 128], F16, name="wtt")
            hot = const.tile([128, B * 192], F16, name="hot")
            bt = const.tile([128, B], F32, name="bt")
            # startup order: transform needs wt+bias+ft0 first; ho is only
            # needed by the second stage and is loaded in block-aligned
            # chunks so early blocks don't wait on the whole table
            # wt in two chunks so pair 0's weights arrive first
            nc.sync.dma_start(out=wtt[:, 0:256], in_=wt[:, 0:256])
            nc.sync.dma_start(out=bt[:], in_=bia[:])

            PRE = 3            # ft prefetch depth (ft tag bufs)
            ftts = []
            for q in range(min(PRE, PB)):
                ftt = work.tile([128, 768], F16, name="ftt", tag="ft",
                                bufs=PRE)
                nc.sync.dma_start(out=ftt[:], in_=ft[q])
                ftts.append(ftt)
                if q == 0:
                    nc.sync.dma_start(out=wtt[:, 256:PB * 128],
                                      in_=wt[:, 256:PB * 128])
                    nc.sync.dma_start(out=hot[:, 0:4 * 192],
                                      in_=ho[:, 0:4 * 192])
            for c0 in range(4, B, 8):
                c1 = min(c0 + 8, B)
                nc.sync.dma_start(out=hot[:, c0 * 192:c1 * 192],
                                  in_=ho[:, c0 * 192:c1 * 192])



            # --- software-pipelined emission -------------------------------
            # The second stage of block b is emitted after the transforms of
            # block b+1, so by the time the PE reaches i01(b) the products
            # p[b] were computed while it streamed block b+1's transforms —
            # the PE never sits on the relu->mul chain of the current block.
            ps2s = {}
            ps = {}

            def pair_transforms(q):
                # interleave the two blocks' transform matmuls so adjacent
                # PE instructions target different array row-halves
                # (tile_position (0,0) vs (64,0)) and can overlap
                ftt = ftts[q]
                ps1s = [psum.tile([128, 2, 512], F32, name="ps1", tag="ps1")
                        for _ in range(2)]
                for s in range(2):
                    for par in range(2):
                        pl, ph = 64 * par, 64 * (par + 1)
                        nc.tensor.matmul(
                            out=ps1s[par][:, s, 0:384],
                            lhsT=wtt[pl:ph, q * 128:(q + 1) * 128],
                            rhs=ftt[pl:ph, 384 * s:384 * (s + 1)],
                            start=True, stop=True,
                        )
                return ps1s

            def elem(b, ps1):
                q, par = divmod(b, 2)
                # relu+bias split by segment across both engines
                t = work.tile([128, 2, 384], F16, name="t", tag="t", bufs=3)
                nc.scalar.activation(out=t[:, 0], in_=ps1[:, 0, 0:384],
                                     func=Relu, bias=bt[:, b:b + 1],
                                     scale=1.0)
                nc.vector.tensor_scalar(
                    out=t[:, 1], in0=ps1[:, 1, 0:384],
                    scalar1=bt[:, b:b + 1], scalar2=0.0,
                    op0=Add, op1=Max)

                p = work.tile([128, 3, 2, 128], F16, name="p", tag="p",
                              bufs=3)
                nc.vector.tensor_mul(out=p[:, 0], in0=t[:, :, 128:256],
                                     in1=t[:, :, 256:384])
                nc.gpsimd.tensor_mul(out=p[:, 1], in0=t[:, :, 0:128],
                                     in1=t[:, :, 256:384])
                nc.gpsimd.tensor_mul(out=p[:, 2], in0=t[:, :, 0:128],
                                     in1=t[:, :, 128:256])
                ps[b] = p

            def pair_second(qq):
                ps2s[qq] = psum.tile([128, 2, 512], F32, name="ps2",
                                     tag="ps2")
                ps2 = ps2s[qq]
                for par in range(2):
                    b = 2 * qq + par
                    base = b * 192
                    p = ps.pop(b)
                    nc.tensor.matmul(
                        out=ps2[:, par, :],
                        lhsT=hot[:, base:base + 128],
                        rhs=p[:, 0:2].rearrange("r i h e -> r (i h e)"),
                        start=True, stop=True)
                    nc.tensor.matmul(
                        out=ps2[0:64, par, 256:512],
                        lhsT=hot[:, base + 128:base + 192],
                        rhs=p[:, 2].rearrange("r h e -> r (h e)"),
                        start=True, stop=True)

            def pair_out(q):
                ps2 = ps2s.pop(q)
                if q + PRE < PB:
                    nftt = work.tile([128, 768], F16, name="ftt", tag="ft",
                                     bufs=PRE)
                    nc.sync.dma_start(out=nftt[:], in_=ft[q + PRE])
                    ftts.append(nftt)
                mt = work.tile([128, 1024], F16, name="mt", tag="m")
                if q == PB - 1:
                    # tail: per-bank copy+store so the even block's results
                    # stream out while the odd block is still computing
                    nc.scalar.activation(out=mt[:, 0:512], in_=ps2[:, 0],
                                         func=Copy, bias=0.0, scale=1.0)
                    nc.sync.dma_start(out=msgs[q, :, 0:512], in_=mt[:, 0:512])
                    nc.vector.tensor_copy(out=mt[:, 512:1024], in_=ps2[:, 1])
                    nc.sync.dma_start(out=msgs[q, :, 512:1024],
                                      in_=mt[:, 512:1024])
                else:
                    nc.scalar.activation(
                        out=mt[:], in_=ps2[:].rearrange("p a b -> p (a b)"),
                        func=Copy, bias=0.0, scale=1.0)
                    nc.sync.dma_start(out=msgs[q], in_=mt[:])

            for q in range(PB):
                ps1s = pair_transforms(q)
                elem(2 * q, ps1s[0])
                elem(2 * q + 1, ps1s[1])
                if q >= 1:
                    pair_second(q - 1)
                    pair_out(q - 1)
            pair_second(PB - 1)
            pair_out(PB - 1)
    nc.finalize()
    return nc


def _get_program(B):
    if B not in _prog_cache:
        _prog_cache[B] = _build_program(B)
    return _prog_cache[B]


def _prepare(x, nodes, fact, params, bias_p, ho_params, ho_bias):
    """Host-side: sort by id, build per-block packed arrays (fp16)."""
    N, L = nodes.shape
    E = fact.shape[0]
    R = params.shape[2]
    NP = params.shape[0]           # 169
    MA = int(round(NP ** 0.5))     # 13

    ids = (x[fact[:, 0], 1] * MA + x[fact[:, 0], 2]).astype(np.int64)   # [E]
    perm = np.argsort(ids, kind="stable")
    ids_s = ids[perm]
    fact_s = fact[perm].astype(np.int64)                                 # [E,3]

    counts = np.bincount(ids_s, minlength=NP)                            # [NP]
    nblk = (counts + _BLK - 1) // _BLK                                   # [NP]
    blk_ids = np.repeat(np.arange(NP), nblk)                             # [NB]
    NB = int(blk_ids.shape[0])
    B = (NB + _NCORES - 1) // _NCORES
    if B % 2:
        B += 1
    NB8 = B * _NCORES
    PB = B // 2
    Q = NB8 // 2
    blk_ids = np.concatenate([blk_ids, np.zeros(NB8 - NB, np.int64)])

    # slot -> sorted-edge-position map (-1 = padding)
    padded = nblk * _BLK
    pad_off = np.concatenate([[0], np.cumsum(padded)])
    off = np.concatenate([[0], np.cumsum(counts)])
    total = int(pad_off[-1])
    t_of = np.repeat(np.arange(NP), padded)
    jloc = np.arange(total) - pad_off[t_of]
    src = np.where(jloc < counts[t_of], off[t_of] + jloc, -1)
    src = np.concatenate([src, np.full(NB8 * _BLK - total, -1, np.int64)])
    valid = src >= 0

    # gather features per slot (fp16)
    nf = nodes[fact_s].astype(np.float16)                                # [E,3,L]
    featp = np.zeros((NB8 * _BLK, 3, L), np.float16)
    featp[valid] = nf[src[valid]]

    # ft: [Q, 128, 768], partition = 64*par + l, col = s*384 + i*128 + e
    ftp = (
        featp.reshape(NB8, 2, _SEG, 3, L)        # b, s, e, i, l
        .transpose(0, 4, 1, 3, 2)                # b, l, s, i, e
        .reshape(Q, 2, L, 768)                   # q, par, l, col
        .reshape(Q, 128, 768)
    )

    # wt: [8, 128, PB*128], partition = 64*par + l, col = q*128 + r
    W = params[blk_ids].astype(np.float16)                               # [NB8,L,R]
    wtp = (
        W.reshape(_NCORES, PB, 2, L, R)          # c, q, par, l, r
        .transpose(0, 2, 3, 1, 4)                # c, par, l, q, r
        .reshape(_NCORES, 128, PB * R)
    )

    # ho: [8, 128, B*192], row r, col = b*192 + i*64 + l
    hop = (
        ho_params[:, blk_ids].astype(np.float16)  # i, b, r, l
        .transpose(1, 2, 0, 3)                    # b, r, i, l
        .reshape(_NCORES, B, R, 192)
        .transpose(0, 2, 1, 3)                    # c, r, b, (i l)
        .reshape(_NCORES, R, B * 192)
    )

    biasT = bias_p[blk_ids, 0].astype(np.float32)                        # [NB8,R]
    biasT = biasT.reshape(_NCORES, B, R).transpose(0, 2, 1)              # [8,R,B]

    return dict(ftp=np.ascontiguousarray(ftp),
                wtp=np.ascontiguousarray(wtp),
                hop=np.ascontiguousarray(hop),
                biasT=np.ascontiguousarray(biasT),
                B=B, NB8=NB8, Q=Q,
                src=src, valid=valid, fact_s=fact_s, ids_s=ids_s,
                N=N, E=E, L=L)


def _postprocess(msgs_all, prep, ho_bias):
    """Decode per-slot messages, add host-side b2, segment-sum into node_msg."""
    NB8, N, E, L = prep["NB8"], prep["N"], prep["E"], prep["L"]
    Q = prep["Q"]
    src, valid, fact_s, ids_s = prep["src"], prep["valid"], prep["fact_s"], prep["ids_s"]

    # msgs_all [Q, 128, 1024]: row = 64*rh + l, col = par*512 + c
    # block b=2q+par: msg0 = [rh=0, c 0:256], msg2 = [rh=0, c 256:512],
    #                 msg1 = [rh=1, c 256:512]; [rh=1, c 0:256] is garbage.
    arr = msgs_all.astype(np.float32).reshape(Q, 2, 64, 2, 512)  # q,rh,l,par,c
    m0 = arr[:, 0, :, :, 0:256]          # q,l,par,(s e)
    m1 = arr[:, 1, :, :, 256:512]
    m2 = arr[:, 0, :, :, 256:512]
    msgs_i = np.stack([m0, m1, m2], axis=2)       # q,l,i,par,(s e)
    slots = (
        msgs_i.reshape(Q, L, 3, 2, 2, _SEG)       # q,l,i,par,s,e
        .transpose(0, 3, 4, 5, 2, 1)              # q,par,s,e,i,l
        .reshape(NB8 * _BLK, 3, L)
    )

    msg_e = np.empty((E, 3, L), np.float32)
    msg_e[src[valid]] = slots[valid]

    # fold in the second bias (linear in the segment-sum)
    msg_e += ho_bias[:, ids_s, 0].astype(np.float32).transpose(1, 0, 2)  # [E,3,L]

    idx_all = fact_s.T.reshape(-1)                                       # [3E]
    val_all = msg_e.transpose(1, 0, 2).reshape(-1, L)                    # [3E,L]
    order = np.argsort(idx_all, kind="stable")
    idx_sorted = idx_all[order]
    val_sorted = val_all[order]
    uniq, starts = np.unique(idx_sorted, return_index=True)
    sums = np.add.reduceat(val_sorted, starts, axis=0)
    out = np.zeros((N, L), np.float32)
    out[uniq] = sums
    return out


def _run_device(prep, trace=False, trace_kwargs=None):
    from concourse.bass_utils import run_bass_kernel_spmd

    B = prep["B"]
    PB = B // 2
    nc = _get_program(B)
    in_maps = []
    for c in range(_NCORES):
        in_maps.append({
            "ft": prep["ftp"][c * PB:(c + 1) * PB],
            "wt": prep["wtp"][c],
            "ho": prep["hop"][c],
            "bia": prep["biasT"][c],
        })
    kwargs = {}
    if trace:
        kwargs["trace"] = True
        if trace_kwargs:
            kwargs.update(trace_kwargs)
    res = run_bass_kernel_spmd(nc, in_maps, list(range(_NCORES)), **kwargs)
    msgs_all = np.concatenate([res.results[c]["msgs"] for c in range(_NCORES)],
                              axis=0)
    return msgs_all, res


def kernel(x, nodes, fact, fact_dim, params, bias_p, ho_params, ho_bias,
           _trace=False, _trace_kwargs=None):
    x = np.asarray(x)
    nodes = np.asarray(nodes, dtype=np.float32)
    fact = np.asarray(fact)
    params = np.asarray(params)
    bias_p = np.asarray(bias_p)
    ho_params = np.asarray(ho_params)
    ho_bias = np.asarray(ho_bias)

    prep = _prepare(x, nodes, fact, params, bias_p, ho_params, ho_bias)
    msgs_all, res = _run_device(prep, trace=_trace, trace_kwargs=_trace_kwargs)
    out = _postprocess(msgs_all, prep, ho_bias)
    kernel.last_results = res
    return out
